# revision 1
# baseline (speedup 1.0000x reference)
import numpy as np

import concourse.bacc as bacc
import concourse.mybir as mybir
from concourse.tile import TileContext
from concourse.bass_utils import run_bass_kernel_spmd

BIN_RES = 0.01
NUM_BINS = 512
T0 = 0.0
DECAY = 2.0
N = 131072
NCORES = 8
P = 128
NPC = N // NCORES
TILES = NPC // P
HB = BIN_RES / 2.0
C1 = float(np.sqrt(0.5 / np.pi))


def _build(spx, spy, spz):
    nc = bacc.Bacc(None, target_bir_lowering=False)
    f32 = mybir.dt.float32
    AF = mybir.ActivationFunctionType
    OP = mybir.AluOpType

    pk = nc.dram_tensor("pk", [P, 7 * TILES], f32, kind="ExternalInput")
    rfull = nc.dram_tensor("rfull", [P, NUM_BINS], f32, kind="ExternalInput")
    ird = nc.dram_tensor("ird", [1, NUM_BINS], f32, kind="ExternalInput")
    hist = nc.dram_tensor("hist", [1, NUM_BINS], f32, kind="ExternalOutput")

    with TileContext(nc) as tc:
        with tc.tile_pool(name="const", bufs=1) as const, \
             tc.tile_pool(name="work", bufs=3) as work, \
             tc.tile_pool(name="psum", bufs=1, space="PSUM") as psum:
            pkt = const.tile([P, 7 * TILES], f32)
            nc.gpsimd.dma_start(out=pkt, in_=pk[:, :])
            rt = const.tile([P, NUM_BINS], f32)
            nc.gpsimd.dma_start(out=rt, in_=rfull[:, :])
            irdt = const.tile([1, NUM_BINS], f32)
            nc.gpsimd.dma_start(out=irdt, in_=ird[:, :])

            T = TILES
            mx = pkt[:, 0 * T:1 * T]
            my = pkt[:, 1 * T:2 * T]
            mz = pkt[:, 2 * T:3 * T]
            col = pkt[:, 3 * T:4 * T]
            cf = pkt[:, 4 * T:5 * T]
            opa = pkt[:, 5 * T:6 * T]
            pas = pkt[:, 6 * T:7 * T]

            spc = const.tile([P, 4], f32)
            nc.vector.memset(spc[:, 0:1], -spx)
            nc.vector.memset(spc[:, 1:2], -spy)
            nc.vector.memset(spc[:, 2:3], -spz)
            nc.vector.memset(spc[:, 3:4], 1e-12)
            dx2 = const.tile([P, T], f32)
            nc.scalar.activation(out=dx2, in_=mx, func=AF.Square, bias=spc[:, 0:1])
            dy2 = const.tile([P, T], f32)
            nc.scalar.activation(out=dy2, in_=my, func=AF.Square, bias=spc[:, 1:2])
            dz2 = const.tile([P, T], f32)
            nc.scalar.activation(out=dz2, in_=mz, func=AF.Square, bias=spc[:, 2:3])
            r0sq = const.tile([P, T], f32)
            nc.vector.tensor_tensor(out=r0sq, in0=dx2, in1=dy2, op=OP.add)
            nc.vector.tensor_tensor(out=r0sq, in0=r0sq, in1=dz2, op=OP.add)
            lnr = const.tile([P, T], f32)
            nc.scalar.activation(out=lnr, in_=r0sq, func=AF.Ln, bias=spc[:, 3:4])
            r0 = const.tile([P, T], f32)
            nc.scalar.activation(out=r0, in_=lnr, func=AF.Exp, scale=0.5)

            isig = const.tile([P, T], f32)
            nc.scalar.activation(out=isig, in_=pas, func=AF.Exp, scale=-1.0)
            nc.vector.tensor_scalar(out=isig, in0=isig, scalar1=1.0 / HB,
                                    scalar2=None, op0=OP.min)
            sig = const.tile([P, T], f32)
            nc.scalar.activation(out=sig, in_=pas, func=AF.Exp)
            nc.vector.tensor_scalar(out=sig, in0=sig, scalar1=HB,
                                    scalar2=None, op0=OP.max)

            ecf = const.tile([P, T], f32)
            nc.scalar.activation(out=ecf, in_=cf, func=AF.Exp)
            om = const.tile([P, T], f32)
            nc.vector.tensor_scalar(out=om, in0=ecf, scalar1=1.0,
                                    scalar2=None, op0=OP.add)
            nc.vector.reciprocal(out=om, in_=om)

            osig = const.tile([P, T], f32)
            nc.scalar.activation(out=osig, in_=opa, func=AF.Exp, scale=-1.0)
            nc.vector.tensor_scalar(out=osig, in0=osig, scalar1=1.0,
                                    scalar2=None, op0=OP.add)
            nc.vector.reciprocal(out=osig, in_=osig)
            col2 = const.tile([P, T], f32)
            nc.vector.tensor_tensor(out=col2, in0=col, in1=col, op=OP.mult)
            inten = const.tile([P, T], f32)
            nc.vector.tensor_tensor(out=inten, in0=osig, in1=col2, op=OP.mult)

            s_all = const.tile([P, T], f32)
            nc.vector.tensor_scalar(out=s_all, in0=isig,
                                    scalar1=float(1.0 / np.sqrt(2.0)),
                                    scalar2=None, op0=OP.mult)
            b_all = const.tile([P, T], f32)
            nc.vector.tensor_tensor(out=b_all, in0=s_all, in1=r0, op=OP.mult)
            nc.vector.tensor_scalar(out=b_all, in0=b_all, scalar1=-1.0,
                                    scalar2=None, op0=OP.mult)

            gam = const.tile([P, T], f32)
            nc.vector.tensor_tensor(out=gam, in0=ecf, in1=sig, op=OP.mult)
            nc.vector.tensor_scalar(out=gam, in0=gam, scalar1=C1,
                                    scalar2=None, op0=OP.mult)
            negthr = const.tile([P, T], f32)
            nc.vector.tensor_tensor(out=negthr, in0=gam, in1=r0, op=OP.subtract)

            isq = const.tile([P, T], f32)
            nc.vector.tensor_tensor(out=isq, in0=isig, in1=isig, op=OP.mult)
            w_all = const.tile([P, T], f32)
            nc.vector.tensor_tensor(out=w_all, in0=om, in1=isq, op=OP.mult)
            nc.vector.tensor_tensor(out=w_all, in0=w_all, in1=inten, op=OP.mult)
            nc.vector.tensor_scalar(out=w_all, in0=w_all,
                                    scalar1=float(HB * np.sqrt(np.pi) / 2.0),
                                    scalar2=None, op0=OP.mult)
            w_bf = const.tile([P, T], mybir.dt.bfloat16)
            nc.vector.tensor_copy(out=w_bf, in_=w_all)

            ps = psum.tile([1, NUM_BINS], f32)
            for t in range(TILES):
                gt = work.tile([P, NUM_BINS], mybir.dt.bfloat16, tag="g")
                nc.scalar.activation(
                    out=gt, in_=rt, func=AF.Derivative_Erf,
                    bias=b_all[:, t:t + 1], scale=s_all[:, t:t + 1])
                ht = work.tile([P, NUM_BINS], mybir.dt.bfloat16, tag="h")
                nc.vector.tensor_scalar(
                    out=ht, in0=rt, scalar1=negthr[:, t:t + 1], scalar2=0.0,
                    op0=OP.add, op1=OP.max)
                pp = work.tile([P, NUM_BINS], mybir.dt.bfloat16, tag="pp")
                nc.vector.tensor_tensor(out=pp, in0=gt, in1=ht, op=OP.mult)
                nc.tensor.matmul(ps, lhsT=w_bf[:, t:t + 1], rhs=pp,
                                 start=(t == 0), stop=(t == TILES - 1))

            hs = const.tile([1, NUM_BINS], f32)
            nc.scalar.copy(out=hs, in_=ps)
            nc.vector.tensor_tensor(out=hs, in0=hs, in1=irdt[0:1, :], op=OP.mult)
            nc.sync.dma_start(out=hist[0:1, :], in_=hs)

    nc.compile()
    return nc


def _shard(inputs):
    means = np.asarray(inputs["means"], dtype=np.float32)
    vid = int(np.asarray(inputs.get("view_id", 0)))
    colours = np.asarray(inputs["colours"], dtype=np.float32)
    coefficients = np.asarray(inputs["coefficients"], dtype=np.float32)
    opacities = np.asarray(inputs["opacities"], dtype=np.float32)
    pre_act_scales = np.asarray(inputs["pre_act_scales"], dtype=np.float32)

    r_ = (np.float32(T0 / 2.0)
          + np.float32(HB) * np.arange(1, 1 + NUM_BINS, dtype=np.float32))
    rd = np.power(r_, np.float32(DECAY), dtype=np.float32)
    ird = (np.float32(1.0) / rd).reshape(1, NUM_BINS)
    rfull = np.broadcast_to(r_, (P, NUM_BINS)).copy()

    def blk(arr, c):
        return np.ascontiguousarray(
            arr[c * NPC:(c + 1) * NPC].reshape(TILES, P).T)

    sig_col = opacities[:, vid]
    in_maps = []
    for c in range(NCORES):
        pk = np.concatenate([
            blk(means[:, 0], c), blk(means[:, 1], c), blk(means[:, 2], c),
            blk(colours[:, 0], c), blk(coefficients[:, 0], c),
            blk(sig_col, c), blk(pre_act_scales[:, 0], c)], axis=1)
        in_maps.append({
            "pk": np.ascontiguousarray(pk, dtype=np.float32),
            "rfull": rfull.astype(np.float32),
            "ird": ird.astype(np.float32),
        })
    return in_maps


def kernel(means, scan_point, colours, coefficients, opacities, pre_act_scales,
           view_id=0, **_unused):
    scan_point = np.asarray(scan_point, dtype=np.float32)
    spx, spy, spz = (float(scan_point[i]) for i in range(3))
    nc = _build(spx, spy, spz)
    in_maps = _shard(dict(means=means, colours=colours,
                          coefficients=coefficients, opacities=opacities,
                          pre_act_scales=pre_act_scales, view_id=view_id))

    res = run_bass_kernel_spmd(nc, in_maps, core_ids=list(range(NCORES)))
    total = np.zeros(NUM_BINS, dtype=np.float64)
    for om in res.results:
        total += om["hist"][0].astype(np.float64)
    return total.astype(np.float32)



# revision 5
# speedup vs baseline: 1.8063x; 1.8063x over previous
import numpy as np

import concourse.bacc as bacc
import concourse.mybir as mybir
from concourse.tile import TileContext
from concourse.bass_utils import run_bass_kernel_spmd

BIN_RES = 0.01
NUM_BINS = 512
HB = BIN_RES / 2.0
DECAY = 2.0
N = 131072
NCORES = 8
P = 128
C1 = float(np.sqrt(0.5 / np.pi))
KS = 4.5
SMAX = 96
R_ = HB * np.arange(1, NUM_BINS + 1, dtype=np.float64)


def _f16(x):
    return np.asarray(x, np.float16).astype(np.float64)


def _prep(means, scan_point, colours, coefficients, opacities, pre_act_scales,
          view_id=0):
    f = np.float64
    means = np.asarray(means, f)
    sp = np.asarray(scan_point, f).reshape(3)
    vid = int(np.asarray(view_id)) if not isinstance(view_id, int) else view_id
    col = np.asarray(colours, f)[:, 0]
    cf = np.asarray(coefficients, f)[:, 0]
    op = np.asarray(opacities, f)[:, vid]
    sig = np.exp(np.asarray(pre_act_scales, f)).mean(axis=1)
    sig = np.maximum(sig, HB)

    d = means - sp[None, :]
    r0 = np.sqrt((d * d).sum(axis=1))
    thr = r0 - C1 * sig * np.exp(cf)
    inten = (1.0 / (1.0 + np.exp(-op))) * col * col
    coeff = 1.0 / (1.0 + np.exp(-cf))
    w = inten * HB * (1.0 - coeff) / (sig * sig) * np.sqrt(np.pi) / 2.0
    s = 1.0 / (sig * np.sqrt(2.0))

    keep = thr <= R_[-1]
    order = np.argsort(r0, kind="stable")
    order = order[keep[order]]
    NK = len(order)
    if NK == 0:
        return None
    TIL = -(-NK // (NCORES * P))
    npad = TIL * NCORES * P - NK
    order = np.concatenate([order, np.repeat(order[-1:], npad)])
    wp = w[order]
    wp[NK:] = 0.0
    r0p, sigp, thrp, sp_ = r0[order], sig[order], thr[order], s[order]

    tiles = []
    rz_cols = []
    lzs = np.empty((5, TIL * P, NCORES), np.float16)
    lws = np.empty((P, 3 * TIL, NCORES), np.float16)
    thrs = np.empty((P, TIL, NCORES), np.float32)
    groups = []
    cur, curw, rzoff = [], 0, 0
    covered = np.zeros(NUM_BINS, bool)
    for t in range(TIL):
        sl = slice(t * NCORES * P, (t + 1) * NCORES * P)
        g_r0, g_sig, g_thr = r0p[sl], sigp[sl], thrp[sl]
        tlo, thi = g_thr.min(), g_thr.max()
        rhi = (g_r0 + KS * g_sig).max()
        lo = max(int(np.searchsorted(R_, tlo)), 0)
        hi = min(int(np.searchsorted(R_, rhi)) + 1, NUM_BINS)
        shi = min(max(int(np.searchsorted(R_, thi, side="left")) + 1, lo + 1),
                  NUM_BINS)
        W, S = hi - lo, shi - lo
        assert 1 <= S <= SMAX and S <= W <= NUM_BINS
        covered[lo:hi] = True
        if curw + W > NUM_BINS:
            groups.append(cur)
            cur, curw = [], 0
        zoff = curw
        cur.append(t)
        curw += W
        tiles.append((lo, W, S, rzoff, zoff, len(groups)))
        rzoff += W
        rbar = R_[(lo + hi) // 2]
        c = R_[lo:hi] - rbar
        ch = _f16(c)
        cl = c - ch
        blk = np.empty((5, W), np.float16)
        blk[0] = ch; blk[1] = cl; blk[2] = ch; blk[3] = 1.0; blk[4] = 1.0
        rz_cols.append(blk)
        sh = _f16(sp_[sl]); slo = sp_[sl] - sh
        b = sp_[sl] * (rbar - r0p[sl])
        bh = _f16(b); bl = b - bh
        w16 = _f16(wp[sl])
        x = w16 * thrp[sl]
        xh = _f16(x); xl = x - xh
        def deal(a):
            return np.asarray(a).reshape(P, NCORES)
        lzs[0, t * P:(t + 1) * P] = deal(sh)
        lzs[1, t * P:(t + 1) * P] = deal(sh)
        lzs[2, t * P:(t + 1) * P] = deal(slo)
        lzs[3, t * P:(t + 1) * P] = deal(bh)
        lzs[4, t * P:(t + 1) * P] = deal(bl)
        lws[:, 3 * t + 0] = deal(w16)
        lws[:, 3 * t + 1] = deal(xh)
        lws[:, 3 * t + 2] = deal(xl)
        thrs[:, t] = deal(thrp[sl]).astype(np.float32)
    groups.append(cur)
    SW = rzoff
    RZ = np.concatenate(rz_cols, axis=1)
    assert RZ.shape == (5, SW)
    NEGR = np.broadcast_to(-R_.astype(np.float32), (P, NUM_BINS)).copy()

    meta = {"TIL": TIL, "SW": SW, "tiles": tiles, "groups": groups,
            "covered": covered}
    in_maps = []
    for cidx in range(NCORES):
        in_maps.append({
            "LZ": np.ascontiguousarray(lzs[:, :, cidx]),
            "RZ": RZ,
            "LW": np.ascontiguousarray(lws[:, :, cidx]),
            "THR": np.ascontiguousarray(thrs[:, :, cidx]),
            "NEGR": NEGR,
        })
    return meta, in_maps


def _build(meta):
    nc = bacc.Bacc(None, target_bir_lowering=False)
    f32, f16 = mybir.dt.float32, mybir.dt.float16
    AF = mybir.ActivationFunctionType
    OP = mybir.AluOpType
    T, SW = meta["TIL"], meta["SW"]
    tiles, groups = meta["tiles"], meta["groups"]

    LZ = nc.dram_tensor("LZ", [5, T * P], f16, kind="ExternalInput")
    RZ = nc.dram_tensor("RZ", [5, SW], f16, kind="ExternalInput")
    LW = nc.dram_tensor("LW", [P, 3 * T], f16, kind="ExternalInput")
    THR = nc.dram_tensor("THR", [P, T], f32, kind="ExternalInput")
    NEGR = nc.dram_tensor("NEGR", [P, NUM_BINS], f32, kind="ExternalInput")
    HOUT = nc.dram_tensor("HOUT", [4, NUM_BINS], f32, kind="ExternalOutput")

    with TileContext(nc) as tc:
        with tc.tile_pool(name="const", bufs=1) as const, \
             tc.tile_pool(name="gpool", bufs=3) as gpool, \
             tc.tile_pool(name="spool", bufs=4) as spool, \
             tc.tile_pool(name="zpool", bufs=4, space="PSUM") as zpool, \
             tc.tile_pool(name="hpool", bufs=1, space="PSUM") as hpool:
            lz = const.tile([5, T * P], f16)
            nc.sync.dma_start(out=lz, in_=LZ[:, :])
            rz = const.tile([5, SW], f16)
            nc.sync.dma_start(out=rz, in_=RZ[:, :])
            lw = const.tile([P, 3 * T], f16)
            nc.sync.dma_start(out=lw, in_=LW[:, :])
            thrt = const.tile([P, T], f32)
            nc.sync.dma_start(out=thrt, in_=THR[:, :])
            negr = const.tile([P, NUM_BINS], f32)
            nc.sync.dma_start(out=negr, in_=NEGR[:, :])
            zl = const.tile([1, 3], f16)
            nc.vector.memset(zl, 0.0)

            ab = hpool.tile([3, NUM_BINS], f32)
            cc = hpool.tile([1, NUM_BINS], f32)
            nc.tensor.matmul(ab, lhsT=zl[0:1, 0:3], rhs=rz[0:1, 0:NUM_BINS],
                             start=True, stop=False, skip_group_check=True)
            nc.tensor.matmul(cc, lhsT=zl[0:1, 0:1], rhs=rz[0:1, 0:NUM_BINS],
                             start=True, stop=False, skip_group_check=True)

            ndone = 0
            for grp in groups:
                GW = sum(tiles[t][1] for t in grp)
                zb = zpool.tile([P, NUM_BINS], f32, tag="z")
                for t in grp:
                    lo, W, S, rzo, zo, _ = tiles[t]
                    nc.tensor.matmul(zb[:, zo:zo + W],
                                     lhsT=lz[:, t * P:(t + 1) * P],
                                     rhs=rz[:, rzo:rzo + W],
                                     start=True, stop=True)
                gb = gpool.tile([P, NUM_BINS], f16, tag="g")
                nc.scalar.activation(out=gb[:, 0:GW], in_=zb[:, 0:GW],
                                     func=AF.Derivative_Erf)
                for t in grp:
                    lo, W, S, rzo, zo, _ = tiles[t]
                    ndone += 1
                    last = ndone == T
                    nc.tensor.matmul(ab[0:3, lo:lo + W],
                                     lhsT=lw[:, 3 * t:3 * t + 3],
                                     rhs=gb[:, zo:zo + W],
                                     start=False, stop=last,
                                     skip_group_check=True)
                    cb = spool.tile([P, SMAX], f16, tag="c")
                    nc.gpsimd.tensor_scalar(out=cb[:, 0:S],
                                            in0=negr[:, lo:lo + S],
                                            scalar1=thrt[:, t:t + 1],
                                            scalar2=0.0,
                                            op0=OP.add, op1=OP.max)
                    pb = spool.tile([P, SMAX], f16, tag="p")
                    nc.gpsimd.tensor_tensor(out=pb[:, 0:S],
                                            in0=gb[:, zo:zo + S],
                                            in1=cb[:, 0:S], op=OP.mult)
                    nc.tensor.matmul(cc[0:1, lo:lo + S],
                                     lhsT=lw[:, 3 * t:3 * t + 1],
                                     rhs=pb[:, 0:S],
                                     start=False, stop=last,
                                     skip_group_check=True)

            hs1 = const.tile([3, NUM_BINS], f32)
            nc.scalar.copy(out=hs1, in_=ab)
            hs2 = const.tile([1, NUM_BINS], f32)
            nc.vector.tensor_copy(out=hs2, in_=cc)
            nc.sync.dma_start(out=HOUT[0:3, :], in_=hs1)
            nc.sync.dma_start(out=HOUT[3:4, :], in_=hs2)

    nc.compile()
    return nc


def _combine(meta, results):
    A = np.zeros(NUM_BINS, np.float64)
    Bh = np.zeros(NUM_BINS, np.float64)
    Bl = np.zeros(NUM_BINS, np.float64)
    C = np.zeros(NUM_BINS, np.float64)
    for om in results:
        hab = np.asarray(om["HOUT"], np.float64)
        A += hab[0]; Bh += hab[1]; Bl += hab[2]; C += hab[3]
    hist = (R_ * A - Bh - Bl + C) / np.power(R_, DECAY)
    hist[~meta["covered"]] = 0.0
    return hist.astype(np.float32)


def kernel(means, scan_point, colours, coefficients, opacities,
           pre_act_scales, view_id=0, **_unused):
    prep = _prep(means, scan_point, colours, coefficients, opacities,
                 pre_act_scales, view_id)
    if prep is None:
        return np.zeros(NUM_BINS, np.float32)
    meta, in_maps = prep
    nc = _build(meta)
    res = run_bass_kernel_spmd(nc, in_maps, core_ids=list(range(NCORES)))
    return _combine(meta, res.results)
